# revision 8
# baseline (speedup 1.0000x reference)
"""Trainium2 Bass kernel for CausalWaveletFieldAttention (v2, fp16).

Shapes (hardcoded): x [B=4, N=4096, D=1024], H=16 heads, HD=64.
Sharding over 8 cores: core c handles (batch b = c//2, half = c%2), i.e.
2048 contiguous sequence rows of one batch.

v2 design (vs v1 which ran the 24-tap wavelet conv as PE diagonal
matmuls): PE does only the dense matmuls; the conv runs entirely on
DVE + GpSimd(Pool) as fused scalar_tensor_tensor chains in fp16
(2x DVE mode), overlapped with the PE gate/coupling phases.

  1. qkv^T: k,v projections in fp16, stationary-outer loop order so one
     LDWEIGHTS serves 4 PSUM banks.
  2. k_mag via PE block-ones reduction of k^2 (fp16), f0 = v * k_mag
  3. pairwise AllGather of f0 (fp16) between the two halves of a batch
  4. conv taps: outputs only [0, SEQ) (width 2048 per tap, not 3072):
     the 1024-col history the d=512/1024 skip taps need is NOT computed
     locally; instead the pair core's conv output cols [1024,2048) are
     AllGathered (second, small collective per chunk).
  5. skip taps fused as scalar_tensor_tensor on DVE
  6. head coupling as dense [1024,1024] fp16 matmul, gate folded in on
     eviction; pg stays in SBUF (no DRAM round-trip)
  7. out = pg @ Wout with PSUM pre-initialized to bout by ScalarE, and
     the result DMAed PSUM -> DRAM directly.
"""

import ml_dtypes
import numpy as np

import concourse.bass as bass
import concourse.mybir as mybir
import concourse.tile as tile
from concourse import bacc
from concourse.bass_utils import run_bass_kernel_spmd

F32 = mybir.dt.float32
F16 = mybir.dt.float16
BF16 = mybir.dt.bfloat16
AF = mybir.ActivationFunctionType
MULT = mybir.AluOpType.mult
ADD = mybir.AluOpType.add

B, N, D, H, HD = 4, 4096, 1024, 16, 64
NCORES = 8
SEQ = N // 2          # 2048 rows per core
KC = D // 128         # 8 chunks of 128 channels
EXTW = 5120           # f0 ext buffer: 1024 zeros | 2048 halo | 2048 own
CVW = 3072            # conv buffer: 1024 gathered ext | 2048 main
D4 = [0.4829629131445341, 0.8365163037378079, 0.2241438680420134, -0.1294095225512604]
N_SCALES = 11
SPARSE_DILATIONS = (512, 1024)
SHIFTS = [0, 1, 2, 3, 4, 6, 8, 12, 16, 24, 32, 48, 64, 96, 128, 192, 256,
          384, 512, 768, 1024, 1536, 2048, 3072]
NT = len(SHIFTS)      # 24 taps
DVE_NTAPS = 19        # taps 0..18 on DVE; rest Act-product + Pool-add

_PROGRAM_CACHE = {}


def _build_program(debug_outputs=False):
    key = bool(debug_outputs)
    if key in _PROGRAM_CACHE:
        return _PROGRAM_CACHE[key]

    nc = bacc.Bacc("TRN2", target_bir_lowering=False, debug=False,
                   num_devices=NCORES)

    # ---- parameters (per-core) ----
    xT = nc.declare_dram_parameter("xT", [D, SEQ], F16, isOutput=False)
    mask = nc.declare_dram_parameter("mask", [128, 1], F32, isOutput=False)
    Wqkv = nc.declare_dram_parameter("Wqkv", [D, 3 * D], F16, isOutput=False)
    bqkvT = nc.declare_dram_parameter("bqkvT", [128, 24], F32, isOutput=False)
    Wgate = nc.declare_dram_parameter("Wgate", [D, D], F16, isOutput=False)
    bgateT = nc.declare_dram_parameter("bgateT", [128, 8], F32, isOutput=False)
    Wout = nc.declare_dram_parameter("Wout", [D, D], BF16, isOutput=False)
    boutB = nc.declare_dram_parameter("boutB", [128, D], F32, isOutput=False)
    Mcoup = nc.declare_dram_parameter("Mcoup", [D, D], BF16, isOutput=False)
    wchan = nc.declare_dram_parameter("wchan", [128, KC, NT], F32, isOutput=False)
    swt = nc.declare_dram_parameter("swt", [128, 2], F32, isOutput=False)
    bo_in = nc.declare_dram_parameter("bo_in", [128, 2], F16, isOutput=False)
    on_in = nc.declare_dram_parameter("on_in", [2, 128], F16, isOutput=False)
    out = nc.declare_dram_parameter("out", [SEQ, D], F32, isOutput=True)

    dbg = {}
    if debug_outputs:
        for name, shape, dt in (("dbg_f0", [D, SEQ], BF16),
                                ("dbg_conv", [D, CVW], BF16),
                                ("dbg_field", [D, SEQ], BF16),
                                ("dbg_gate", [D, SEQ], F32)):
            dbg[name] = nc.declare_dram_parameter(name, shape, dt, isOutput=True)

    # ---- internal DRAM ----
    f0_dram = [nc.dram_tensor(f"f0_dram{c}", [128, SEQ], BF16)
               for c in range(KC)]
    f0_gath = [nc.dram_tensor(f"f0_gath{c}", [2, 128, SEQ], BF16)
               for c in range(KC)]
    cv_dram = [nc.dram_tensor(f"cv_dram{c}", [128, 1024], BF16)
               for c in range(KC)]
    cv_gath = [nc.dram_tensor(f"cv_gath{c}", [2, 128, 1024], BF16)
               for c in range(KC)]
    gate_dram = nc.dram_tensor("gate_dram", [D, SEQ], F32)

    GROUPS = [[0, 1], [2, 3], [4, 5], [6, 7]]

    with tile.TileContext(nc) as tc:
        with (
            tc.tile_pool(name="const", bufs=1) as constp,
            tc.tile_pool(name="p_long", bufs=1) as p_long,
            tc.tile_pool(name="p_cv2", bufs=2) as p_cv2,
        ):
            # ---- constants ----
            wchan_t = constp.tile([128, KC, NT], F32)
            nc.sync.dma_start(wchan_t[:], wchan[:])
            swt_t = constp.tile([128, 2], F32)
            nc.sync.dma_start(swt_t[:], swt[:])
            bqkv_t = constp.tile([128, 24], F32)
            nc.sync.dma_start(bqkv_t[:], bqkvT[:])
            bgate_t = constp.tile([128, 8], F32)
            nc.sync.dma_start(bgate_t[:], bgateT[:])
            mask_t = constp.tile([128, 1], F32)
            nc.sync.dma_start(mask_t[:], mask[:])
            bo_t = constp.tile([128, 2], F16)
            nc.sync.dma_start(bo_t[:], bo_in[:])
            on_t = constp.tile([2, 128], F16)
            nc.sync.dma_start(on_t[:], on_in[:])

            # ---- long-lived SBUF ----
            xm = p_long.tile([128, KC, SEQ], F16, tag="xm")
            field = p_long.tile([128, KC, SEQ], BF16, tag="field")
            exts = [p_long.tile([128, EXTW], BF16, tag=f"ext{i}",
                                name=f"ext{i}") for i in range(2)]
            da = [p_long.tile([128, SEQ], BF16, tag=f"da{i}", name=f"da{i}")
                  for i in range(2)]
            pa = [p_long.tile([128, SEQ], BF16, tag=f"pa{i}", name=f"pa{i}")
                  for i in range(2)]

            for k in range(KC):
                nc.sync.dma_start(xm[:, k, :], xT[k * 128:(k + 1) * 128, :])
            for i in range(2):
                nc.gpsimd.memset(exts[i][:, 0:1024], 0.0)

            # ================= phase A: k,v,f0 then gate =================
            with (
                tc.tile_pool(name="p_strip", bufs=3) as p_strip,
                tc.tile_pool(name="p_work", bufs=2) as p_work,
                tc.tile_pool(name="psp", bufs=6, space="PSUM") as psp,
                tc.tile_pool(name="psps", bufs=1, space="PSUM") as psps,
                tc.tile_pool(name="pspe", bufs=1, space="PSUM") as pspe,
            ):
                def load_strip(col0, src=Wqkv):
                    sr = p_strip.tile([128, KC, 128], F16, tag="strip")
                    nc.sync.dma_start(
                        sr[:],
                        src[:, col0 * 128:(col0 + 1) * 128]
                        .rearrange("(kc p) m -> p kc m", p=128))
                    return sr

                def proj_psums(strip):
                    """4 PSUM banks, stationary-outer so LDWEIGHTS is
                    loaded once per contraction chunk."""
                    pss = [psp.tile([128, 512], F32, tag="ps",
                                    name=f"ps{rb_}") for rb_ in range(4)]
                    for kk in range(KC):
                        for rb in range(4):
                            nc.tensor.matmul(
                                pss[rb][:], strip[:, kk, :],
                                xm[:, kk, rb * 512:(rb + 1) * 512],
                                start=(kk == 0), stop=(kk == KC - 1))
                    return pss

                for c in range(KC):
                    ks = load_strip(8 + c)
                    kps = proj_psums(ks)
                    k2b = p_work.tile([128, SEQ], F16, tag="k2b")
                    for rb in range(4):
                        nc.scalar.activation(k2b[:, rb * 512:(rb + 1) * 512],
                                             kps[rb][:], AF.Square,
                                             bias=bqkv_t[:, 8 + c:9 + c])
                    vs = load_strip(16 + c)
                    vps = proj_psums(vs)
                    vTb = p_work.tile([128, SEQ], F32, tag="vTb")
                    for rb in range(4):
                        nc.scalar.activation(vTb[:, rb * 512:(rb + 1) * 512],
                                             vps[rb][:], AF.Identity,
                                             bias=bqkv_t[:, 16 + c:17 + c])
                    km = p_work.tile([2, SEQ], F16, tag="km")
                    for sb in range(4):
                        pss = psps.tile([2, 512], F32, tag="ps2")
                        nc.tensor.matmul(pss[:], bo_t[:],
                                         k2b[:, sb * 512:(sb + 1) * 512],
                                         start=True, stop=True)
                        nc.scalar.activation(km[:, sb * 512:(sb + 1) * 512],
                                             pss[:], AF.Sqrt)
                    f0b = p_work.tile([128, SEQ], BF16, tag="f0b")
                    for sb in range(4):
                        pse = pspe.tile([128, 512], F32, tag="pse")
                        nc.tensor.matmul(pse[:], on_t[:],
                                         km[:, sb * 512:(sb + 1) * 512],
                                         start=True, stop=True)
                        nc.vector.tensor_mul(f0b[:, sb * 512:(sb + 1) * 512],
                                             vTb[:, sb * 512:(sb + 1) * 512],
                                             pse[:])
                    nc.sync.dma_start(f0_dram[c][:], f0b[:])
                    nc.gpsimd.collective_compute(
                        "AllGather", mybir.AluOpType.bypass,
                        replica_groups=GROUPS,
                        ins=[f0_dram[c][:]], outs=[f0_gath[c][:]])

                # gate = sigmoid(x @ (Wq@Wgate) + b') -> gate_dram (f32)
                for gc in range(KC):
                    gs = load_strip(gc, src=Wgate)
                    gps = proj_psums(gs)
                    for rb in range(4):
                        gsb = p_work.tile([128, 512], F32, tag="gsb")
                        nc.scalar.activation(gsb[:], gps[rb][:], AF.Sigmoid,
                                             bias=bgate_t[:, gc:gc + 1])
                        nc.sync.dma_start(
                            gate_dram[gc * 128:(gc + 1) * 128,
                                      rb * 512:(rb + 1) * 512], gsb[:])

            if debug_outputs:
                for c in range(KC):
                    nc.sync.dma_start(
                        dbg["dbg_f0"][c * 128:(c + 1) * 128, :], f0_dram[c][:])

            # ================= conv on DVE + Pool =================
            # tap s contributes f0[n-s] to output n in [0, SEQ);
            # ext covers f0 positions [-3072, 2048): 1024 zeros, 2048
            # halo (pair core's f0, masked on even cores), 2048 own.
            for c in range(KC):
                ext = exts[c % 2]
                halo = p_cv2.tile([128, SEQ], BF16, tag="halo")
                nc.sync.dma_start(halo[:], f0_gath[c][0, :, :])
                nc.vector.tensor_scalar_mul(ext[:, 1024:3072], halo[:],
                                            mask_t[:, 0:1])
                nc.sync.dma_start(ext[:, 3072:EXTW], f0_dram[c][:])

                dcur = None
                for i in range(DVE_NTAPS):
                    s = SHIFTS[i]
                    src = ext[:, 3072 - s:3072 - s + SEQ]
                    w = wchan_t[:, c, i:i + 1]
                    if dcur is None:
                        dcur = da[0]
                        nc.vector.tensor_scalar_mul(dcur[:], src, w)
                    else:
                        nxt = da[i % 2]
                        nc.vector.scalar_tensor_tensor(
                            nxt[:], src, w, dcur[:], op0=MULT, op1=ADD)
                        dcur = nxt
                # remaining taps: product on ScalarE (per-partition scale),
                # accumulation chain on GpSimd (TensorScalar is not legal on
                # Pool, plain tensor_tensor add is)
                pcur = None
                for j, i in enumerate(range(DVE_NTAPS, NT)):
                    s = SHIFTS[i]
                    src = ext[:, 3072 - s:3072 - s + SEQ]
                    w = wchan_t[:, c, i:i + 1]
                    at = p_cv2.tile([128, SEQ], BF16, tag="at", bufs=3)
                    nc.scalar.activation(at[:], src, AF.Identity, scale=w)
                    if pcur is None:
                        pcur = at
                    else:
                        nxt = pa[j % 2]
                        nc.gpsimd.tensor_add(nxt[:], pcur[:], at[:])
                        pcur = nxt

                convb = p_cv2.tile([128, CVW], BF16, tag="convb")
                nc.gpsimd.tensor_add(convb[:, 1024:CVW], dcur[:], pcur[:])
                # pair-exchange of conv cols [1024,2048) as skip history
                nc.sync.dma_start(cv_dram[c][:], convb[:, 2048:CVW])
                nc.gpsimd.collective_compute(
                    "AllGather", mybir.AluOpType.bypass,
                    replica_groups=GROUPS,
                    ins=[cv_dram[c][:]], outs=[cv_gath[c][:]])
                cvh = p_cv2.tile([128, 1024], BF16, tag="cvh")
                nc.sync.dma_start(cvh[:], cv_gath[c][0, :, :])
                nc.vector.tensor_scalar_mul(convb[:, 0:1024], cvh[:],
                                            mask_t[:, 0:1])
                if debug_outputs:
                    nc.sync.dma_start(
                        dbg["dbg_conv"][c * 128:(c + 1) * 128, :], convb[:])
                tmp = p_cv2.tile([128, SEQ], BF16, tag="tmp")
                nc.vector.scalar_tensor_tensor(
                    tmp[:], convb[:, 512:512 + SEQ], swt_t[:, 0:1],
                    convb[:, 1024:1024 + SEQ], op0=MULT, op1=ADD)
                nc.vector.scalar_tensor_tensor(
                    field[:, c, :], convb[:, 0:SEQ], swt_t[:, 1:2],
                    tmp[:], op0=MULT, op1=ADD)

            if debug_outputs:
                nc.sync.dma_start(dbg["dbg_gate"][:], gate_dram[:])
                with tc.tile_pool(name="p_dbg", bufs=2) as p_dbg:
                    for c in range(KC):
                        dft = p_dbg.tile([128, SEQ], BF16, tag="dft")
                        nc.vector.tensor_copy(dft[:], field[:, c, :])
                        nc.sync.dma_start(
                            dbg["dbg_field"][c * 128:(c + 1) * 128, :],
                            dft[:])

            # ================= coupling + gate -> pgs (SBUF) =============
            with tc.tile_pool(name="p_pg", bufs=1) as p_pg:
              pgs = p_pg.tile([128, KC, SEQ], BF16, tag="pgs")
              with (
                tc.tile_pool(name="p_mr", bufs=2) as p_mr,
                tc.tile_pool(name="p_gt", bufs=4) as p_gt,
                tc.tile_pool(name="pspd", bufs=8, space="PSUM") as pspd,
              ):
                for co in range(KC):
                    mr = p_mr.tile([128, KC, 128], BF16, tag="mr")
                    nc.sync.dma_start(
                        mr[:],
                        Mcoup[:, co * 128:(co + 1) * 128]
                        .rearrange("(kc p) m -> p kc m", p=128))
                    cps = [pspd.tile([128, 512], F32, tag="cps",
                                    name=f"cps{sb_}") for sb_ in range(4)]
                    for ci in range(KC):
                        for sb in range(4):
                            nc.tensor.matmul(
                                cps[sb][:], mr[:, ci, :],
                                field[:, ci, sb * 512:(sb + 1) * 512],
                                start=(ci == 0), stop=(ci == KC - 1))
                    for sb in range(4):
                        gt = p_gt.tile([128, 512], F32, tag="gt")
                        nc.sync.dma_start(
                            gt[:],
                            gate_dram[co * 128:(co + 1) * 128,
                                      sb * 512:(sb + 1) * 512])
                        nc.vector.tensor_mul(
                            pgs[:, co, sb * 512:(sb + 1) * 512],
                            cps[sb][:], gt[:])

              # =============== out = pg @ Wout + bout ====================
              with (
                tc.tile_pool(name="p_wo", bufs=1) as p_wo,
                tc.tile_pool(name="pspo", bufs=4, space="PSUM") as pspo,
              ):
                wo_r = p_wo.tile([128, KC, D], BF16, tag="wo_r")
                for k in range(KC):
                    nc.sync.dma_start(wo_r[:, k, :],
                                      Wout[k * 128:(k + 1) * 128, :])
                bout_t = p_wo.tile([128, D], F32, tag="bout_t")
                nc.sync.dma_start(bout_t[:], boutB[:])
                for st in range(SEQ // 128):
                    pso = [pspo.tile([128, 512], F32, tag="pso",
                                    name=f"pso{cb_}") for cb_ in range(2)]
                    for cb in range(2):
                        nc.scalar.activation(
                            pso[cb][:], bout_t[:, cb * 512:(cb + 1) * 512],
                            AF.Identity)
                    for k in range(KC):
                        for cb in range(2):
                            nc.tensor.matmul(
                                pso[cb][:],
                                pgs[:, k, st * 128:(st + 1) * 128],
                                wo_r[:, k, cb * 512:(cb + 1) * 512],
                                start=False, stop=(k == KC - 1))
                    outb = p_wo.tile([128, D], F32, tag="outb", bufs=2)
                    for cb in range(2):
                        nc.vector.tensor_copy(
                            outb[:, cb * 512:(cb + 1) * 512], pso[cb][:])
                    nc.sync.dma_start(out[st * 128:(st + 1) * 128, :],
                                      outb[:])

    nc.compile()
    _PROGRAM_CACHE[key] = nc
    return nc


def _softmax(a, axis):
    a = a - a.max(axis=axis, keepdims=True)
    e = np.exp(a)
    return e / e.sum(axis=axis, keepdims=True)


def _host_prep(inputs):
    """Build per-core and replicated input tensors from full inputs."""
    x = np.asarray(inputs["x"], np.float32)
    Wqkv = np.ascontiguousarray(np.asarray(inputs["Wqkv"], np.float32))
    bqkv = np.asarray(inputs["bqkv"], np.float32)
    Wout = np.ascontiguousarray(np.asarray(inputs["Wout"], np.float32))
    bout = np.asarray(inputs["bout"], np.float32)
    Wgate = np.ascontiguousarray(np.asarray(inputs["Wgate"], np.float32))
    bgate = np.asarray(inputs["bgate"], np.float32)
    scale_gain = np.asarray(inputs["scale_gain"], np.float64)
    skip_w = np.asarray(inputs["skip_w"], np.float64)
    coupling = np.asarray(inputs["coupling"], np.float64)

    gains = _softmax(scale_gain, axis=0)              # [11, H]
    sw = 1.0 / (1.0 + np.exp(-skip_w))                # [2]
    coup = _softmax(coupling, axis=-1)                # [H, H]

    sidx = {s: i for i, s in enumerate(SHIFTS)}
    wtab = np.zeros((NT, H), np.float64)
    for j in range(N_SCALES):
        d = 1 << j
        for t in range(4):
            wtab[sidx[(3 - t) * d]] += D4[t] * gains[j]
    ch = np.arange(D)
    wchan = np.zeros((128, KC, NT), np.float32)
    for c in range(KC):
        heads = (ch[c * 128:(c + 1) * 128] // HD)
        wchan[:, c, :] = wtab[:, heads].T.astype(np.float32)

    Mc = np.zeros((D, D), np.float32)
    idx = np.arange(HD)
    for i in range(H):
        for j in range(H):
            Mc[j * HD + idx, i * HD + idx] = coup[i, j]

    # fold the q projection into the gate: gate = sigmoid(x @ (Wq@Wgate) + b')
    Wq = Wqkv[:, :D].astype(np.float64)
    Wqg = np.ascontiguousarray(
        (Wq @ Wgate.astype(np.float64)).astype(np.float16))
    bg_f = (bqkv[:D].astype(np.float64) @ Wgate.astype(np.float64)
            + bgate.astype(np.float64)).astype(np.float32)

    bqkvT = bqkv.reshape(24, 128).T.copy()            # [128, 24]
    bgateT = bg_f.reshape(8, 128).T.copy()            # [128, 8]
    boutB = np.broadcast_to(bout, (128, D)).copy()
    swt = np.broadcast_to(sw.astype(np.float32), (128, 2)).copy()
    bo = np.zeros((128, 2), np.float16)
    bo[0:64, 0] = 1.0
    bo[64:128, 1] = 1.0
    on = np.zeros((2, 128), np.float16)
    on[0, 0:64] = 1.0
    on[1, 64:128] = 1.0

    shared = dict(Wqkv=Wqkv.astype(np.float16), bqkvT=bqkvT, Wgate=Wqg,
                  bgateT=bgateT, Wout=Wout.astype(ml_dtypes.bfloat16), boutB=boutB,
                  Mcoup=Mc.astype(ml_dtypes.bfloat16), wchan=wchan, swt=swt,
                  bo_in=bo, on_in=on)
    in_maps = []
    for c in range(NCORES):
        b, half = c // 2, c % 2
        g0 = half * SEQ
        xTc = np.ascontiguousarray(x[b, g0:g0 + SEQ, :].T.astype(np.float16))
        m = np.full((128, 1), float(half), np.float32)
        in_maps.append(dict(xT=xTc, mask=m, **shared))
    return in_maps


def run_cores(inputs, debug_outputs=False, trace=False):
    nc = _build_program(debug_outputs=debug_outputs)
    in_maps = _host_prep(inputs)
    res = run_bass_kernel_spmd(nc, in_maps, list(range(NCORES)), trace=trace)
    return res


def kernel(**inputs) -> np.ndarray:
    res = run_cores(inputs)
    out = np.empty((B, N, D), np.float32)
    for c in range(NCORES):
        b, half = c // 2, c % 2
        out[b, half * SEQ:(half + 1) * SEQ, :] = res.results[c]["out"]
    return out
